# revision 12
# baseline (speedup 1.0000x reference)
"""Trainium2 Bass kernel for nn_CompetitionHGT (2-layer HGT, 60k nodes, 800k edges).

Strategy: destination-sharded edge parallelism across 8 NeuronCores.
 - Nodes padded to 128-blocks, reordered core-major; core k owns 49 b-blocks + 10 c-blocks.
 - Edges sorted by destination, grouped per (core, dst-block, relation), padded to
   128-edge chunks with uniform chunk counts across cores (single SPMD program).
 - Per layer: replicated k|v projection into a bf16 table (per-relation transform
   folded into q-side weights: q.(k@A) = (q@A^T).k, with p_rel*scale folded in);
   edge phase gathers kv rows by src via indirect DMA, builds per-chunk one-hot
   (dst-within-block) matrices, expands q' per edge via PE matmul, computes
   exp(logits), payload v*exp, and scatter-adds via one-hot matmuls into PSUM per
   (block, relation); epilogue normalizes by the joint softmax sum, applies m_rel,
   gelu, typed output linear and gated skip. Chunks are processed in groups of 4
   to amortize vector-engine instruction overhead.
 - Between layers: AllGather of transposed h blocks (bf16).
"""
import sys
if '/opt/trn_rl_repo' not in sys.path:
    sys.path.insert(0, '/opt/trn_rl_repo')

import numpy as np
import ml_dtypes
from contextlib import ExitStack

import concourse.bass as bass
import concourse.bacc as bacc
import concourse.tile as tile
from concourse import mybir
from concourse.bass_utils import run_bass_kernel_spmd
from concourse.masks import make_identity

BF = ml_dtypes.bfloat16
F32, BF16, I32 = mybir.dt.float32, mybir.dt.bfloat16, mybir.dt.int32
AOT = mybir.AluOpType
ACTF = mybir.ActivationFunctionType

NB, NC = 50000, 10000
NBP, NCP = 50176, 10240
BB, CB = NBP // 128, NCP // 128          # 392, 80
NCORES = 8
BBC, CBC = BB // NCORES, CB // NCORES    # 49, 10
NBLK = BBC + CBC                         # 59 blocks per core
OWN = NBLK * 128                         # 7552 rows per core
NTOT = NCORES * OWN                      # 60416
P = 128
G = 4                                    # chunks per group

LAST_RESULT = None


def _g_b(n):
    n = np.asarray(n)
    return (n // (BBC * 128)) * OWN + (n % (BBC * 128))


def _g_c(m):
    m = np.asarray(m)
    return (m // (CBC * 128)) * OWN + BBC * 128 + (m % (CBC * 128))


def _bf(x):
    return np.ascontiguousarray(np.asarray(x, np.float32).astype(BF))


def _prep_edges(e_bb_src, e_bb_dst, e_bc_src, e_bc_dst, e_cb_src, e_cb_dst):
    rels = [(_g_b, 'b', e_bb_src, e_bb_dst),
            (_g_b, 'c', e_bc_src, e_bc_dst),
            (_g_c, 'b', e_cb_src, e_cb_dst)]
    buckets = [[[None] * 3 for _ in range(NBLK)] for _ in range(NCORES)]
    for r, (gsrc_fn, dspace, src, dst) in enumerate(rels):
        order = np.argsort(dst, kind='stable')
        src, dst = np.asarray(src)[order], np.asarray(dst)[order]
        gsrc = gsrc_fn(src)
        if dspace == 'b':
            core = dst // (BBC * 128)
            lblk = (dst % (BBC * 128)) // 128
        else:
            core = dst // (CBC * 128)
            lblk = BBC + (dst % (CBC * 128)) // 128
        dloc = (dst % 128).astype(np.float32)
        key = core.astype(np.int64) * 64 + lblk
        bounds = np.searchsorted(key, np.arange(NCORES * 64 + 64))
        for c in range(NCORES):
            for lb in range(NBLK):
                if (lb < BBC) != (dspace == 'b'):
                    continue
                k = c * 64 + lb
                buckets[c][lb][r] = (gsrc[bounds[k]:bounds[k + 1]],
                                     dloc[bounds[k]:bounds[k + 1]])

    blk_rels = []
    for lb in range(BBC):
        blk_rels += [(lb, 0), (lb, 2)]
    for lb in range(BBC, NBLK):
        blk_rels.append((lb, 1))
    counts = []
    for (lb, r) in blk_rels:
        mx = max(len(buckets[c][lb][r][0]) for c in range(NCORES))
        counts.append(max(1, (mx + P - 1) // P))
    nch = sum(counts)

    gsrc_a = np.zeros((NCORES, nch, P), np.int32)
    dloc_a = np.full((NCORES, nch, P), 255.0, np.float32)
    for c in range(NCORES):
        col = 0
        for (lb, r), cc in zip(blk_rels, counts):
            gs, dl = buckets[c][lb][r]
            n = len(gs)
            fg = np.zeros(cc * P, np.int32); fg[:n] = gs
            fd = np.full(cc * P, 255.0, np.float32); fd[:n] = dl
            gsrc_a[c, col:col + cc] = fg.reshape(cc, P)
            dloc_a[c, col:col + cc] = fd.reshape(cc, P)
            col += cc
    return (blk_rels, counts,
            np.ascontiguousarray(gsrc_a.transpose(0, 2, 1)),
            np.ascontiguousarray(dloc_a.transpose(0, 2, 1).astype(BF)))


def _fold_weights(inp):
    W = {}
    for L, (H, ch) in (('1', (4, 128)), ('2', (1, 64))):
        D = ch // H
        Wkqv = np.asarray(inp[f'l{L}_Wkqv'], np.float32)
        arel = np.asarray(inp[f'l{L}_arel'], np.float32)
        mrel = np.asarray(inp[f'l{L}_mrel'], np.float32)
        prel = np.asarray(inp[f'l{L}_prel'], np.float32)
        scale = np.float32(1.0 / np.sqrt(np.float32(D)))
        Wk, Wq, Wv = Wkqv[:, :, :ch], Wkqv[:, :, ch:2 * ch], Wkqv[:, :, 2 * ch:]
        W[f'kv{L}'] = np.concatenate([Wk, Wv], axis=2)
        qp, mb = [], []
        for r, dt in ((0, 0), (1, 1), (2, 0)):
            blk = np.zeros((ch, ch), np.float32)
            mblk = np.zeros((ch, ch), np.float32)
            for h in range(H):
                blk[h * D:(h + 1) * D, h * D:(h + 1) * D] = arel[r, h].T * (prel[r, h] * scale)
                mblk[h * D:(h + 1) * D, h * D:(h + 1) * D] = mrel[r, h]
            qp.append(Wq[dt] @ blk)
            mb.append(mblk)
        W[f'qp{L}'] = np.stack(qp)
        W[f'mb{L}'] = np.stack(mb)
        beta = 1.0 / (1.0 + np.exp(-np.asarray(inp[f'l{L}_skip'], np.float64)))
        W[f'beta{L}'] = beta.astype(np.float32)
        Wout = np.asarray(inp[f'l{L}_Wout'], np.float32)
        W[f'out{L}'] = Wout * beta[:, None, None].astype(np.float32) if L == '1' else Wout
    return W


def _build_nc(blk_rels, counts):
    nch = sum(counts)
    per_block = {}
    idx = 0
    for (lb, r), cc in zip(blk_rels, counts):
        per_block.setdefault(lb, []).append((r, idx, cc))
        idx += cc

    nc = bacc.Bacc("TRN2", target_bir_lowering=False, debug=False, num_devices=NCORES)

    xT = nc.dram_tensor("xT", [NTOT, 128], BF16, kind="ExternalInput")
    xTown = nc.dram_tensor("xTown", [OWN, 128], BF16, kind="ExternalInput")
    xrows = nc.dram_tensor("xrows", [OWN, 128], F32, kind="ExternalInput")
    gsrc_d = nc.dram_tensor("gsrc", [P, nch], I32, kind="ExternalInput")
    dloc_d = nc.dram_tensor("dloc", [P, nch], BF16, kind="ExternalInput")
    wkv1 = nc.dram_tensor("wkv1", [2 * 128, 256], BF16, kind="ExternalInput")
    wqp1 = nc.dram_tensor("wqp1", [3 * 128, 128], BF16, kind="ExternalInput")
    wmb1 = nc.dram_tensor("wmb1", [3 * 128, 128], BF16, kind="ExternalInput")
    wout1 = nc.dram_tensor("wout1", [2 * 128, 128], BF16, kind="ExternalInput")
    wkv2 = nc.dram_tensor("wkv2", [2 * 128, 128], BF16, kind="ExternalInput")
    wqp2 = nc.dram_tensor("wqp2", [3 * 128, 64], BF16, kind="ExternalInput")
    wmb2 = nc.dram_tensor("wmb2", [3 * 64, 64], BF16, kind="ExternalInput")
    wout2 = nc.dram_tensor("wout2", [2 * 64, 64], BF16, kind="ExternalInput")
    out_d = nc.dram_tensor("out", [OWN, 64], F32, kind="ExternalOutput")

    with tile.TileContext(nc) as tc:
        with ExitStack() as ctx:
            const = ctx.enter_context(tc.tile_pool(name="const", bufs=1))
            sb = ctx.enter_context(tc.tile_pool(name="sb", bufs=6))
            sbk = ctx.enter_context(tc.tile_pool(name="sbk", bufs=6))
            sbg = ctx.enter_context(tc.tile_pool(name="sbg", bufs=18))
            pp_nd = ctx.enter_context(tc.tile_pool(name="ppnd", bufs=3, space="PSUM"))
            pp_acc = ctx.enter_context(tc.tile_pool(name="ppacc", bufs=3, space="PSUM"))
            pp_epi = ctx.enter_context(tc.tile_pool(name="ppepi", bufs=2, space="PSUM"))
            dram = ctx.enter_context(tc.tile_pool(name="dram", bufs=1, space="DRAM"))

            kv1 = dram.tile([NTOT, 256], BF16, tag="kv1")
            kv2 = dram.tile([NTOT, 128], BF16, tag="kv2")
            hT_own = dram.tile([OWN, 128], BF16, tag="hTown")
            hT_glob = dram.tile([NTOT, 128], BF16, tag="hTglob", addr_space="Shared")

            iota_i = const.tile([P, G * P], I32, tag="iotai")
            nc.gpsimd.iota(iota_i[:], pattern=[[0, G], [1, P]], base=0,
                           channel_multiplier=0)
            iota4 = const.tile([P, G, P], BF16, tag="iota4")
            nc.vector.tensor_copy(out=iota4[:].rearrange("p k d -> p (k d)"),
                                  in_=iota_i[:])
            ident = const.tile([P, P], BF16, tag="ident")
            make_identity(nc, ident[:])

            def wtile(dram_t, rows, cols, tag, row0=0):
                t = const.tile([rows, cols], BF16, tag=tag)
                nc.sync.dma_start(out=t[:], in_=dram_t[row0:row0 + rows, :])
                return t

            wkv1_t = [wtile(wkv1, 128, 256, f"wkv1{t}", t * 128) for t in range(2)]
            wqp1_t = [wtile(wqp1, 128, 128, f"wqp1{r}", r * 128) for r in range(3)]
            wmb1_t = [wtile(wmb1, 128, 128, f"wmb1{r}", r * 128) for r in range(3)]
            wout1_t = [wtile(wout1, 128, 128, f"wout1{t}", t * 128) for t in range(2)]
            wkv2_t = [wtile(wkv2, 128, 128, f"wkv2{t}", t * 128) for t in range(2)]
            wqp2_t = [wtile(wqp2, 128, 64, f"wqp2{r}", r * 128) for r in range(3)]
            wmb2_t = [wtile(wmb2, 64, 64, f"wmb2{r}", r * 64) for r in range(3)]
            wout2_t = [wtile(wout2, 64, 64, f"wout2{t}", t * 64) for t in range(2)]

            gsrc_sb = const.tile([P, nch], I32, tag="gsrcsb")
            nc.sync.dma_start(out=gsrc_sb[:], in_=gsrc_d[:])
            dloc_sb = const.tile([P, nch], BF16, tag="dlocsb")
            nc.sync.dma_start(out=dloc_sb[:], in_=dloc_d[:])

            def blk_type(lb):
                return 0 if lb < BBC else 1

            def node_phase(src_dram, kv_tab, wkv_t, kvcols):
                NG = 512 // kvcols   # blocks per psum bank (2 for L1, 4 for L2)
                for gi, g0 in enumerate(range(0, NCORES * NBLK, NG)):
                    xt = sb.tile([P, NG, 128], BF16, tag="nd_x")
                    ldq = nc.sync if gi % 2 == 0 else nc.scalar
                    ldq.dma_start(
                        out=xt[:].rearrange("p k d -> p (k d)"),
                        in_=src_dram[g0 * 128:(g0 + NG) * 128, :].rearrange(
                            "(k p) d -> p k d", p=P))
                    ps = pp_nd.tile([P, NG, kvcols], F32, tag="nd_ps", space="PSUM")
                    for j in range(NG):
                        nc.tensor.matmul(out=ps[:, j, :], lhsT=xt[:, j, :],
                                         rhs=wkv_t[blk_type((g0 + j) % NBLK)][:],
                                         start=True, stop=True)
                    kvb = sb.tile([P, NG, kvcols], BF16, tag="nd_kv")
                    nc.vector.tensor_copy(out=kvb[:].rearrange("p k d -> p (k d)"),
                                          in_=ps[:].rearrange("p k d -> p (k d)"))
                    nc.scalar.dma_start(
                        out=kv_tab[g0 * 128:(g0 + NG) * 128, :].rearrange(
                            "(k p) d -> p k d", p=P),
                        in_=kvb[:])

            def edge_phase(layer, kv_tab, src_own, wqp_t, wmb_t, wout_t, H, ch):
                D = ch // H
                F = ch + H
                for lb in range(NBLK):
                    typ = blk_type(lb)
                    xo = sb.tile([P, 128], BF16, tag="e_xo")
                    nc.sync.dma_start(out=xo[:], in_=src_own[lb * 128:(lb + 1) * 128, :])
                    accs = []
                    for (r, cstart, cc) in per_block[lb]:
                        qp_ps = pp_nd.tile([P, 512], F32, tag="nd_ps", space="PSUM")
                        nc.tensor.matmul(out=qp_ps[:, :ch], lhsT=xo[:], rhs=wqp_t[r][:],
                                         start=True, stop=True)
                        qp_sb = sb.tile([P, ch], BF16, tag="e_qp")
                        nc.vector.tensor_copy(out=qp_sb[:], in_=qp_ps[:, :ch])
                        acc = pp_acc.tile([P, F], F32, tag="acc", space="PSUM")
                        gsizes = [G] * (cc // G) + ([cc % G] if cc % G else [])
                        ngrp = len(gsizes)
                        gcol = 0
                        for gi, gs in enumerate(gsizes):
                            col = cstart + gcol
                            gcol += gs
                            kv4 = sbg.tile([P, G, 2 * ch], BF16, tag="e_kv")
                            for j in range(gs):
                                nc.gpsimd.indirect_dma_start(
                                    out=kv4[:, j, :], out_offset=None, in_=kv_tab[:],
                                    in_offset=bass.IndirectOffsetOnAxis(
                                        ap=gsrc_sb[:, col + j:col + j + 1], axis=0))
                            ohT = sbk.tile([P, G, P], BF16, tag="e_oh")
                            nc.vector.tensor_tensor(
                                out=ohT[:, :gs, :], in0=iota4[:, :gs, :],
                                in1=dloc_sb[:, col:col + gs].to_broadcast([P, gs, P]),
                                op=AOT.is_equal)
                            oh_ps = pp_nd.tile([P, G, P], BF16, tag="nd_ps", space="PSUM")
                            for j in range(gs):
                                nc.tensor.transpose(out=oh_ps[:, j, :], in_=ohT[:, j, :],
                                                    identity=ident[:])
                            oh_sb = sbk.tile([P, G, P], BF16, tag="e_ohs")
                            nc.vector.tensor_copy(
                                out=oh_sb[:, :gs, :], in_=oh_ps[:, :gs, :])
                            qe_ps = pp_nd.tile([P, G, ch], F32, tag="nd_ps", space="PSUM")
                            for j in range(gs):
                                nc.tensor.matmul(out=qe_ps[:, j, :], lhsT=oh_sb[:, j, :],
                                                 rhs=qp_sb[:], start=True, stop=True)
                            prod = sbk.tile([P, G, ch], BF16, tag="e_pr")
                            nc.vector.tensor_tensor(
                                out=prod[:, :gs, :], in0=qe_ps[:, :gs, :],
                                in1=kv4[:, :gs, :ch], op=AOT.mult)
                            lgt = sbk.tile([P, G, H], F32, tag="e_lg")
                            nc.vector.reduce_sum(
                                out=lgt[:, :gs, :],
                                in_=prod[:, :gs, :].rearrange("p k (h d) -> p k h d", h=H),
                                axis=mybir.AxisListType.X)
                            pay = sbk.tile([P, G, F], BF16, tag="e_pay")
                            nc.scalar.activation(out=pay[:, :gs, ch:F], in_=lgt[:, :gs, :],
                                                 func=ACTF.Exp)
                            nc.vector.tensor_tensor(
                                out=pay[:, :gs, :ch].rearrange("p k (h d) -> p k h d", h=H),
                                in0=kv4[:, :gs, ch:2 * ch].rearrange("p k (h d) -> p k h d", h=H),
                                in1=pay[:, :gs, ch:F].to_broadcast([P, gs, H, D]),
                                op=AOT.mult)
                            for j in range(gs):
                                nc.tensor.matmul(
                                    out=acc[:], lhsT=ohT[:, j, :], rhs=pay[:, j, :],
                                    start=(gi == 0 and j == 0),
                                    stop=(gi == ngrp - 1 and j == gs - 1))
                        accs.append((r, acc))
                    # epilogue
                    s = sb.tile([P, H], F32, tag="ep_s")
                    nc.vector.tensor_copy(out=s[:], in_=accs[0][1][:, ch:F])
                    if len(accs) > 1:
                        nc.vector.tensor_tensor(out=s[:], in0=accs[1][1][:, ch:F],
                                                in1=s[:], op=AOT.add)
                    nc.vector.tensor_scalar(out=s[:], in0=s[:], scalar1=1e-16,
                                            scalar2=None, op0=AOT.add)
                    recip = sb.tile([P, H], F32, tag="ep_r")
                    nc.vector.reciprocal(out=recip[:], in_=s[:])
                    aggT = pp_epi.tile([ch, P], F32, tag="ep", space="PSUM")
                    for j, (r, acc) in enumerate(accs):
                        vn = sb.tile([P, ch], BF16, tag="ep_vn")
                        nc.vector.tensor_tensor(
                            out=vn[:].rearrange("p (h d) -> p h d", h=H),
                            in0=acc[:, :ch].rearrange("p (h d) -> p h d", h=H),
                            in1=recip[:].to_broadcast([P, H, D]), op=AOT.mult)
                        vT_ps = pp_epi.tile([ch, P], BF16, tag="ep", space="PSUM")
                        nc.tensor.transpose(out=vT_ps[:], in_=vn[:], identity=ident[:])
                        vT = sb.tile([ch, P], BF16, tag="ep_vts")
                        nc.vector.tensor_copy(out=vT[:], in_=vT_ps[:])
                        nc.tensor.matmul(out=aggT[:], lhsT=wmb_t[r][:], rhs=vT[:],
                                         start=(j == 0), stop=(j == len(accs) - 1))
                    gl = sb.tile([ch, P], BF16, tag="ep_gl")
                    nc.scalar.activation(out=gl[:], in_=aggT[:], func=ACTF.Gelu)
                    ops = pp_epi.tile([P, ch if layer == 1 else 64], F32, tag="ep",
                                      space="PSUM")
                    nc.tensor.matmul(out=ops[:], lhsT=gl[:], rhs=wout_t[typ][:],
                                     start=True, stop=True)
                    if layer == 1:
                        xr = sb.tile([P, 128], F32, tag="ep_xr")
                        nc.sync.dma_start(out=xr[:], in_=xrows[lb * 128:(lb + 1) * 128, :])
                        hrow = sb.tile([P, 128], BF16, tag="ep_hr")
                        nc.vector.tensor_tensor(out=hrow[:], in0=ops[:], in1=xr[:],
                                                op=AOT.add)
                        hT_ps = pp_epi.tile([P, P], BF16, tag="ep", space="PSUM")
                        nc.tensor.transpose(out=hT_ps[:], in_=hrow[:], identity=ident[:])
                        hT = sb.tile([P, P], BF16, tag="ep_hts")
                        nc.vector.tensor_scalar(out=hT[:], in0=hT_ps[:], scalar1=0.0,
                                                scalar2=None, op0=AOT.max)
                        nc.scalar.dma_start(out=hT_own[lb * 128:(lb + 1) * 128, :], in_=hT[:])
                    else:
                        fin = sb.tile([P, 64], F32, tag="ep_fin")
                        nc.vector.tensor_copy(out=fin[:], in_=ops[:])
                        nc.scalar.dma_start(out=out_d[lb * 128:(lb + 1) * 128, :], in_=fin[:])

            # ---- layer 1 ----
            node_phase(xT, kv1, wkv1_t, 256)
            edge_phase(1, kv1, xTown, wqp1_t, wmb1_t, wout1_t, 4, 128)
            # ---- exchange h ----
            nc.gpsimd.collective_compute(
                "AllGather", AOT.bypass, replica_groups=[list(range(NCORES))],
                ins=[hT_own.opt()], outs=[hT_glob.opt()])
            # ---- layer 2 ----
            node_phase(hT_glob, kv2, wkv2_t, 128)
            edge_phase(2, kv2, hT_own, wqp2_t, wmb2_t, wout2_t, 1, 64)

    nc.compile()
    return nc


_CACHE = {}


def kernel(**inputs):
    inp = {k: np.asarray(v) for k, v in inputs.items()}

    blk_rels, counts, gsrc, dloc = _prep_edges(
        inp['e_bb_src'], inp['e_bb_dst'], inp['e_bc_src'], inp['e_bc_dst'],
        inp['e_cb_src'], inp['e_cb_dst'])
    W = _fold_weights(inp)

    xg = np.zeros((NTOT, 128), np.float32)
    xb_pad = np.zeros((NBP, 128), np.float32); xb_pad[:NB] = inp['x_b']
    xc_pad = np.zeros((NCP, 128), np.float32); xc_pad[:NC] = inp['x_c']
    xg[_g_b(np.arange(NBP))] = xb_pad
    xg[_g_c(np.arange(NCP))] = xc_pad
    xg_bf = _bf(xg)
    xT = np.ascontiguousarray(
        xg_bf.reshape(NCORES * NBLK, 128, 128).transpose(0, 2, 1)).reshape(NTOT, 128)
    one_minus_beta = np.empty(NTOT, np.float32)
    for c in range(NCORES):
        one_minus_beta[c * OWN:c * OWN + BBC * 128] = 1.0 - W['beta1'][0]
        one_minus_beta[c * OWN + BBC * 128:(c + 1) * OWN] = 1.0 - W['beta1'][1]
    xrows_folded = (xg_bf.astype(np.float32) * one_minus_beta[:, None]).astype(np.float32)

    key = tuple(counts)
    if key not in _CACHE:
        _CACHE[key] = _build_nc(blk_rels, counts)
    nc = _CACHE[key]

    wk = {
        'wkv1': _bf(W['kv1'].reshape(2 * 128, 256)),
        'wqp1': _bf(W['qp1'].reshape(3 * 128, 128)),
        'wmb1': _bf(W['mb1'].reshape(3 * 128, 128)),
        'wout1': _bf(W['out1'].reshape(2 * 128, 128)),
        'wkv2': _bf(W['kv2'].reshape(2 * 128, 128)),
        'wqp2': _bf(W['qp2'].reshape(3 * 128, 64)),
        'wmb2': _bf(W['mb2'].reshape(3 * 64, 64)),
        'wout2': _bf(W['out2'].reshape(2 * 64, 64)),
    }
    in_maps = []
    for c in range(NCORES):
        m = dict(wk)
        m['xT'] = xT
        m['xTown'] = np.ascontiguousarray(xT[c * OWN:(c + 1) * OWN])
        m['xrows'] = np.ascontiguousarray(xrows_folded[c * OWN:(c + 1) * OWN])
        m['gsrc'] = gsrc[c]
        m['dloc'] = dloc[c]
        in_maps.append(m)

    import os
    trace = bool(os.environ.get("HGT_TRACE"))
    res = run_bass_kernel_spmd(nc, in_maps, core_ids=list(range(NCORES)), trace=trace)
    global LAST_RESULT
    LAST_RESULT = res

    out_full = np.concatenate([res.results[c]['out'] for c in range(NCORES)], axis=0)
    o_b = out_full[_g_b(np.arange(NB))].astype(np.float32)
    o_c = out_full[_g_c(np.arange(NC))].astype(np.float32)
    return o_b, o_c


# revision 13
# speedup vs baseline: 1.0350x; 1.0350x over previous
"""Trainium2 Bass kernel for nn_CompetitionHGT (2-layer HGT, 60k nodes, 800k edges).

Strategy: destination-sharded edge parallelism across 8 NeuronCores.
 - Nodes padded to 128-blocks, reordered core-major; core k owns 49 b-blocks + 10 c-blocks.
 - Edges sorted by destination, grouped per (core, dst-block, relation), padded to
   128-edge chunks with uniform chunk counts across cores (single SPMD program).
 - Per layer: replicated k|v projection into a bf16 table (per-relation transform
   folded into q-side weights: q.(k@A) = (q@A^T).k, with p_rel*scale folded in);
   edge phase gathers kv rows by src via indirect DMA, builds per-chunk one-hot
   (dst-within-block) matrices, expands q' per edge via PE matmul, computes
   exp(logits), payload v*exp, and scatter-adds via one-hot matmuls into PSUM per
   (block, relation); epilogue normalizes by the joint softmax sum, applies m_rel,
   gelu, typed output linear and gated skip. Chunks are processed in groups of 4
   to amortize vector-engine instruction overhead.
 - Between layers: AllGather of transposed h blocks (bf16).
"""
import sys
if '/opt/trn_rl_repo' not in sys.path:
    sys.path.insert(0, '/opt/trn_rl_repo')

import numpy as np
import ml_dtypes
from contextlib import ExitStack

import concourse.bass as bass
import concourse.bacc as bacc
import concourse.tile as tile
from concourse import mybir
from concourse.bass_utils import run_bass_kernel_spmd
from concourse.masks import make_identity

BF = ml_dtypes.bfloat16
F32, BF16, I32 = mybir.dt.float32, mybir.dt.bfloat16, mybir.dt.int32
AOT = mybir.AluOpType
ACTF = mybir.ActivationFunctionType

NB, NC = 50000, 10000
NBP, NCP = 50176, 10240
BB, CB = NBP // 128, NCP // 128          # 392, 80
NCORES = 8
BBC, CBC = BB // NCORES, CB // NCORES    # 49, 10
NBLK = BBC + CBC                         # 59 blocks per core
OWN = NBLK * 128                         # 7552 rows per core
NTOT = NCORES * OWN                      # 60416
P = 128
G = 4                                    # chunks per group

LAST_RESULT = None


def _g_b(n):
    n = np.asarray(n)
    return (n // (BBC * 128)) * OWN + (n % (BBC * 128))


def _g_c(m):
    m = np.asarray(m)
    return (m // (CBC * 128)) * OWN + BBC * 128 + (m % (CBC * 128))


def _bf(x):
    return np.ascontiguousarray(np.asarray(x, np.float32).astype(BF))


def _prep_edges(e_bb_src, e_bb_dst, e_bc_src, e_bc_dst, e_cb_src, e_cb_dst):
    rels = [(_g_b, 'b', e_bb_src, e_bb_dst),
            (_g_b, 'c', e_bc_src, e_bc_dst),
            (_g_c, 'b', e_cb_src, e_cb_dst)]
    buckets = [[[None] * 3 for _ in range(NBLK)] for _ in range(NCORES)]
    for r, (gsrc_fn, dspace, src, dst) in enumerate(rels):
        order = np.argsort(dst, kind='stable')
        src, dst = np.asarray(src)[order], np.asarray(dst)[order]
        gsrc = gsrc_fn(src)
        if dspace == 'b':
            core = dst // (BBC * 128)
            lblk = (dst % (BBC * 128)) // 128
        else:
            core = dst // (CBC * 128)
            lblk = BBC + (dst % (CBC * 128)) // 128
        dloc = (dst % 128).astype(np.float32)
        key = core.astype(np.int64) * 64 + lblk
        bounds = np.searchsorted(key, np.arange(NCORES * 64 + 64))
        for c in range(NCORES):
            for lb in range(NBLK):
                if (lb < BBC) != (dspace == 'b'):
                    continue
                k = c * 64 + lb
                buckets[c][lb][r] = (gsrc[bounds[k]:bounds[k + 1]],
                                     dloc[bounds[k]:bounds[k + 1]])

    blk_rels = []
    for lb in range(BBC):
        blk_rels += [(lb, 0), (lb, 2)]
    for lb in range(BBC, NBLK):
        blk_rels.append((lb, 1))
    counts = []
    for (lb, r) in blk_rels:
        mx = max(len(buckets[c][lb][r][0]) for c in range(NCORES))
        counts.append(max(1, (mx + P - 1) // P))
    nch = sum(counts)

    gsrc_a = np.zeros((NCORES, nch, P), np.int32)
    dloc_a = np.full((NCORES, nch, P), 255.0, np.float32)
    for c in range(NCORES):
        col = 0
        for (lb, r), cc in zip(blk_rels, counts):
            gs, dl = buckets[c][lb][r]
            n = len(gs)
            fg = np.zeros(cc * P, np.int32); fg[:n] = gs
            fd = np.full(cc * P, 255.0, np.float32); fd[:n] = dl
            gsrc_a[c, col:col + cc] = fg.reshape(cc, P)
            dloc_a[c, col:col + cc] = fd.reshape(cc, P)
            col += cc
    return (blk_rels, counts,
            np.ascontiguousarray(gsrc_a.transpose(0, 2, 1)),
            np.ascontiguousarray(dloc_a.transpose(0, 2, 1).astype(BF)))


def _fold_weights(inp):
    W = {}
    for L, (H, ch) in (('1', (4, 128)), ('2', (1, 64))):
        D = ch // H
        Wkqv = np.asarray(inp[f'l{L}_Wkqv'], np.float32)
        arel = np.asarray(inp[f'l{L}_arel'], np.float32)
        mrel = np.asarray(inp[f'l{L}_mrel'], np.float32)
        prel = np.asarray(inp[f'l{L}_prel'], np.float32)
        scale = np.float32(1.0 / np.sqrt(np.float32(D)))
        Wk, Wq, Wv = Wkqv[:, :, :ch], Wkqv[:, :, ch:2 * ch], Wkqv[:, :, 2 * ch:]
        W[f'kv{L}'] = np.concatenate([Wk, Wv], axis=2)
        qp, mb = [], []
        for r, dt in ((0, 0), (1, 1), (2, 0)):
            blk = np.zeros((ch, ch), np.float32)
            mblk = np.zeros((ch, ch), np.float32)
            for h in range(H):
                blk[h * D:(h + 1) * D, h * D:(h + 1) * D] = arel[r, h].T * (prel[r, h] * scale)
                mblk[h * D:(h + 1) * D, h * D:(h + 1) * D] = mrel[r, h]
            qp.append(Wq[dt] @ blk)
            mb.append(mblk)
        W[f'qp{L}'] = np.stack(qp)
        W[f'mb{L}'] = np.stack(mb)
        beta = 1.0 / (1.0 + np.exp(-np.asarray(inp[f'l{L}_skip'], np.float64)))
        W[f'beta{L}'] = beta.astype(np.float32)
        Wout = np.asarray(inp[f'l{L}_Wout'], np.float32)
        W[f'out{L}'] = Wout * beta[:, None, None].astype(np.float32) if L == '1' else Wout
    return W


def _build_nc(blk_rels, counts):
    nch = sum(counts)
    per_block = {}
    idx = 0
    for (lb, r), cc in zip(blk_rels, counts):
        per_block.setdefault(lb, []).append((r, idx, cc))
        idx += cc

    nc = bacc.Bacc("TRN2", target_bir_lowering=False, debug=False, num_devices=NCORES)

    xT = nc.dram_tensor("xT", [NTOT, 128], BF16, kind="ExternalInput")
    xTown = nc.dram_tensor("xTown", [OWN, 128], BF16, kind="ExternalInput")
    xrows = nc.dram_tensor("xrows", [OWN, 128], F32, kind="ExternalInput")
    gsrc_d = nc.dram_tensor("gsrc", [P, nch], I32, kind="ExternalInput")
    dloc_d = nc.dram_tensor("dloc", [P, nch], BF16, kind="ExternalInput")
    wkv1 = nc.dram_tensor("wkv1", [2 * 128, 256], BF16, kind="ExternalInput")
    wqp1 = nc.dram_tensor("wqp1", [3 * 128, 128], BF16, kind="ExternalInput")
    wmb1 = nc.dram_tensor("wmb1", [3 * 128, 128], BF16, kind="ExternalInput")
    wout1 = nc.dram_tensor("wout1", [2 * 128, 128], BF16, kind="ExternalInput")
    wkv2 = nc.dram_tensor("wkv2", [2 * 128, 128], BF16, kind="ExternalInput")
    wqp2 = nc.dram_tensor("wqp2", [3 * 128, 64], BF16, kind="ExternalInput")
    wmb2 = nc.dram_tensor("wmb2", [3 * 64, 64], BF16, kind="ExternalInput")
    wout2 = nc.dram_tensor("wout2", [2 * 64, 64], BF16, kind="ExternalInput")
    out_d = nc.dram_tensor("out", [OWN, 64], F32, kind="ExternalOutput")

    with tile.TileContext(nc) as tc:
        with ExitStack() as ctx:
            const = ctx.enter_context(tc.tile_pool(name="const", bufs=1))
            sb = ctx.enter_context(tc.tile_pool(name="sb", bufs=6))
            sbk = ctx.enter_context(tc.tile_pool(name="sbk", bufs=6))
            sbg = ctx.enter_context(tc.tile_pool(name="sbg", bufs=18))
            pp_nd = ctx.enter_context(tc.tile_pool(name="ppnd", bufs=3, space="PSUM"))
            pp_acc = ctx.enter_context(tc.tile_pool(name="ppacc", bufs=3, space="PSUM"))
            pp_epi = ctx.enter_context(tc.tile_pool(name="ppepi", bufs=2, space="PSUM"))
            dram = ctx.enter_context(tc.tile_pool(name="dram", bufs=1, space="DRAM"))

            kv1 = dram.tile([NTOT, 256], BF16, tag="kv1")
            kv2 = dram.tile([NTOT, 128], BF16, tag="kv2")
            hT_own = dram.tile([OWN, 128], BF16, tag="hTown")
            hT_glob = dram.tile([NTOT, 128], BF16, tag="hTglob", addr_space="Shared")

            iota_i = const.tile([P, G * P], I32, tag="iotai")
            nc.gpsimd.iota(iota_i[:], pattern=[[0, G], [1, P]], base=0,
                           channel_multiplier=0)
            iota4 = const.tile([P, G, P], BF16, tag="iota4")
            nc.vector.tensor_copy(out=iota4[:].rearrange("p k d -> p (k d)"),
                                  in_=iota_i[:])
            ident = const.tile([P, P], BF16, tag="ident")
            make_identity(nc, ident[:])

            def wtile(dram_t, rows, cols, tag, row0=0):
                t = const.tile([rows, cols], BF16, tag=tag)
                nc.sync.dma_start(out=t[:], in_=dram_t[row0:row0 + rows, :])
                return t

            wkv1_t = [wtile(wkv1, 128, 256, f"wkv1{t}", t * 128) for t in range(2)]
            wqp1_t = [wtile(wqp1, 128, 128, f"wqp1{r}", r * 128) for r in range(3)]
            wmb1_t = [wtile(wmb1, 128, 128, f"wmb1{r}", r * 128) for r in range(3)]
            wout1_t = [wtile(wout1, 128, 128, f"wout1{t}", t * 128) for t in range(2)]
            wkv2_t = [wtile(wkv2, 128, 128, f"wkv2{t}", t * 128) for t in range(2)]
            wqp2_t = [wtile(wqp2, 128, 64, f"wqp2{r}", r * 128) for r in range(3)]
            wmb2_t = [wtile(wmb2, 64, 64, f"wmb2{r}", r * 64) for r in range(3)]
            wout2_t = [wtile(wout2, 64, 64, f"wout2{t}", t * 64) for t in range(2)]

            gsrc_sb = const.tile([P, nch], I32, tag="gsrcsb")
            nc.sync.dma_start(out=gsrc_sb[:], in_=gsrc_d[:])
            dloc_sb = const.tile([P, nch], BF16, tag="dlocsb")
            nc.sync.dma_start(out=dloc_sb[:], in_=dloc_d[:])

            def blk_type(lb):
                return 0 if lb < BBC else 1

            def node_phase(src_dram, kv_tab, wkv_t, kvcols):
                NG = 512 // kvcols   # blocks per psum bank (2 for L1, 4 for L2)
                for g0 in range(0, NCORES * NBLK, NG):
                    xt = sb.tile([P, NG, 128], BF16, tag="nd_x")
                    nc.sync.dma_start(
                        out=xt[:].rearrange("p k d -> p (k d)"),
                        in_=src_dram[g0 * 128:(g0 + NG) * 128, :].rearrange(
                            "(k p) d -> p k d", p=P))
                    ps = pp_nd.tile([P, NG, kvcols], F32, tag="nd_ps", space="PSUM")
                    for j in range(NG):
                        nc.tensor.matmul(out=ps[:, j, :], lhsT=xt[:, j, :],
                                         rhs=wkv_t[blk_type((g0 + j) % NBLK)][:],
                                         start=True, stop=True)
                    kvb = sb.tile([P, NG, kvcols], BF16, tag="nd_kv")
                    nc.vector.tensor_copy(out=kvb[:].rearrange("p k d -> p (k d)"),
                                          in_=ps[:].rearrange("p k d -> p (k d)"))
                    nc.scalar.dma_start(
                        out=kv_tab[g0 * 128:(g0 + NG) * 128, :].rearrange(
                            "(k p) d -> p k d", p=P),
                        in_=kvb[:])

            def edge_phase(layer, kv_tab, src_own, wqp_t, wmb_t, wout_t, H, ch):
                D = ch // H
                F = ch + H
                for lb in range(NBLK):
                    typ = blk_type(lb)
                    xo = sb.tile([P, 128], BF16, tag="e_xo")
                    nc.sync.dma_start(out=xo[:], in_=src_own[lb * 128:(lb + 1) * 128, :])
                    accs = []
                    for (r, cstart, cc) in per_block[lb]:
                        qp_ps = pp_nd.tile([P, 512], F32, tag="nd_ps", space="PSUM")
                        nc.tensor.matmul(out=qp_ps[:, :ch], lhsT=xo[:], rhs=wqp_t[r][:],
                                         start=True, stop=True)
                        qp_sb = sb.tile([P, ch], BF16, tag="e_qp")
                        nc.vector.tensor_copy(out=qp_sb[:], in_=qp_ps[:, :ch])
                        acc = pp_acc.tile([P, F], F32, tag="acc", space="PSUM")
                        gsizes = [G] * (cc // G) + ([cc % G] if cc % G else [])
                        ngrp = len(gsizes)
                        gcol = 0
                        for gi, gs in enumerate(gsizes):
                            col = cstart + gcol
                            gcol += gs
                            kv4 = sbg.tile([P, G, 2 * ch], BF16, tag="e_kv")
                            for j in range(gs):
                                nc.gpsimd.indirect_dma_start(
                                    out=kv4[:, j, :], out_offset=None, in_=kv_tab[:],
                                    in_offset=bass.IndirectOffsetOnAxis(
                                        ap=gsrc_sb[:, col + j:col + j + 1], axis=0))
                            ohT = sbk.tile([P, G, P], BF16, tag="e_oh")
                            nc.vector.tensor_tensor(
                                out=ohT[:, :gs, :], in0=iota4[:, :gs, :],
                                in1=dloc_sb[:, col:col + gs].to_broadcast([P, gs, P]),
                                op=AOT.is_equal)
                            oh_ps = pp_nd.tile([P, G, P], BF16, tag="nd_ps", space="PSUM")
                            for j in range(gs):
                                nc.tensor.transpose(out=oh_ps[:, j, :], in_=ohT[:, j, :],
                                                    identity=ident[:])
                            oh_sb = sbk.tile([P, G, P], BF16, tag="e_ohs")
                            nc.vector.tensor_copy(
                                out=oh_sb[:, :gs, :], in_=oh_ps[:, :gs, :])
                            qe_ps = pp_nd.tile([P, G, ch], F32, tag="nd_ps", space="PSUM")
                            for j in range(gs):
                                nc.tensor.matmul(out=qe_ps[:, j, :], lhsT=oh_sb[:, j, :],
                                                 rhs=qp_sb[:], start=True, stop=True)
                            prod = sbk.tile([P, G, ch], BF16, tag="e_pr")
                            nc.vector.tensor_tensor(
                                out=prod[:, :gs, :], in0=qe_ps[:, :gs, :],
                                in1=kv4[:, :gs, :ch], op=AOT.mult)
                            lgt = sbk.tile([P, G, H], F32, tag="e_lg")
                            nc.vector.reduce_sum(
                                out=lgt[:, :gs, :],
                                in_=prod[:, :gs, :].rearrange("p k (h d) -> p k h d", h=H),
                                axis=mybir.AxisListType.X)
                            pay = sbk.tile([P, G, F], BF16, tag="e_pay")
                            nc.scalar.activation(out=pay[:, :gs, ch:F], in_=lgt[:, :gs, :],
                                                 func=ACTF.Exp)
                            nc.vector.tensor_tensor(
                                out=pay[:, :gs, :ch].rearrange("p k (h d) -> p k h d", h=H),
                                in0=kv4[:, :gs, ch:2 * ch].rearrange("p k (h d) -> p k h d", h=H),
                                in1=pay[:, :gs, ch:F].to_broadcast([P, gs, H, D]),
                                op=AOT.mult)
                            for j in range(gs):
                                nc.tensor.matmul(
                                    out=acc[:], lhsT=ohT[:, j, :], rhs=pay[:, j, :],
                                    start=(gi == 0 and j == 0),
                                    stop=(gi == ngrp - 1 and j == gs - 1))
                        accs.append((r, acc))
                    # epilogue
                    s = sb.tile([P, H], F32, tag="ep_s")
                    nc.vector.tensor_copy(out=s[:], in_=accs[0][1][:, ch:F])
                    if len(accs) > 1:
                        nc.vector.tensor_tensor(out=s[:], in0=accs[1][1][:, ch:F],
                                                in1=s[:], op=AOT.add)
                    nc.vector.tensor_scalar(out=s[:], in0=s[:], scalar1=1e-16,
                                            scalar2=None, op0=AOT.add)
                    recip = sb.tile([P, H], F32, tag="ep_r")
                    nc.vector.reciprocal(out=recip[:], in_=s[:])
                    aggT = pp_epi.tile([ch, P], F32, tag="ep", space="PSUM")
                    for j, (r, acc) in enumerate(accs):
                        vn = sb.tile([P, ch], BF16, tag="ep_vn")
                        nc.vector.tensor_tensor(
                            out=vn[:].rearrange("p (h d) -> p h d", h=H),
                            in0=acc[:, :ch].rearrange("p (h d) -> p h d", h=H),
                            in1=recip[:].to_broadcast([P, H, D]), op=AOT.mult)
                        vT_ps = pp_epi.tile([ch, P], BF16, tag="ep", space="PSUM")
                        nc.tensor.transpose(out=vT_ps[:], in_=vn[:], identity=ident[:])
                        vT = sb.tile([ch, P], BF16, tag="ep_vts")
                        nc.vector.tensor_copy(out=vT[:], in_=vT_ps[:])
                        nc.tensor.matmul(out=aggT[:], lhsT=wmb_t[r][:], rhs=vT[:],
                                         start=(j == 0), stop=(j == len(accs) - 1))
                    gl = sb.tile([ch, P], BF16, tag="ep_gl")
                    nc.scalar.activation(out=gl[:], in_=aggT[:], func=ACTF.Gelu)
                    ops = pp_epi.tile([P, ch if layer == 1 else 64], F32, tag="ep",
                                      space="PSUM")
                    nc.tensor.matmul(out=ops[:], lhsT=gl[:], rhs=wout_t[typ][:],
                                     start=True, stop=True)
                    if layer == 1:
                        xr = sb.tile([P, 128], F32, tag="ep_xr")
                        nc.sync.dma_start(out=xr[:], in_=xrows[lb * 128:(lb + 1) * 128, :])
                        hrow = sb.tile([P, 128], BF16, tag="ep_hr")
                        nc.vector.tensor_tensor(out=hrow[:], in0=ops[:], in1=xr[:],
                                                op=AOT.add)
                        hT_ps = pp_epi.tile([P, P], BF16, tag="ep", space="PSUM")
                        nc.tensor.transpose(out=hT_ps[:], in_=hrow[:], identity=ident[:])
                        hT = sb.tile([P, P], BF16, tag="ep_hts")
                        nc.vector.tensor_scalar(out=hT[:], in0=hT_ps[:], scalar1=0.0,
                                                scalar2=None, op0=AOT.max)
                        nc.scalar.dma_start(out=hT_own[lb * 128:(lb + 1) * 128, :], in_=hT[:])
                    else:
                        fin = sb.tile([P, 64], F32, tag="ep_fin")
                        nc.vector.tensor_copy(out=fin[:], in_=ops[:])
                        nc.scalar.dma_start(out=out_d[lb * 128:(lb + 1) * 128, :], in_=fin[:])

            # ---- layer 1 ----
            node_phase(xT, kv1, wkv1_t, 256)
            edge_phase(1, kv1, xTown, wqp1_t, wmb1_t, wout1_t, 4, 128)
            # ---- exchange h ----
            nc.gpsimd.collective_compute(
                "AllGather", AOT.bypass, replica_groups=[list(range(NCORES))],
                ins=[hT_own.opt()], outs=[hT_glob.opt()])
            # ---- layer 2 ----
            node_phase(hT_glob, kv2, wkv2_t, 128)
            edge_phase(2, kv2, hT_own, wqp2_t, wmb2_t, wout2_t, 1, 64)

    nc.compile()
    return nc


_CACHE = {}


def kernel(**inputs):
    inp = {k: np.asarray(v) for k, v in inputs.items()}

    blk_rels, counts, gsrc, dloc = _prep_edges(
        inp['e_bb_src'], inp['e_bb_dst'], inp['e_bc_src'], inp['e_bc_dst'],
        inp['e_cb_src'], inp['e_cb_dst'])
    W = _fold_weights(inp)

    xg = np.zeros((NTOT, 128), np.float32)
    xb_pad = np.zeros((NBP, 128), np.float32); xb_pad[:NB] = inp['x_b']
    xc_pad = np.zeros((NCP, 128), np.float32); xc_pad[:NC] = inp['x_c']
    xg[_g_b(np.arange(NBP))] = xb_pad
    xg[_g_c(np.arange(NCP))] = xc_pad
    xg_bf = _bf(xg)
    xT = np.ascontiguousarray(
        xg_bf.reshape(NCORES * NBLK, 128, 128).transpose(0, 2, 1)).reshape(NTOT, 128)
    one_minus_beta = np.empty(NTOT, np.float32)
    for c in range(NCORES):
        one_minus_beta[c * OWN:c * OWN + BBC * 128] = 1.0 - W['beta1'][0]
        one_minus_beta[c * OWN + BBC * 128:(c + 1) * OWN] = 1.0 - W['beta1'][1]
    xrows_folded = (xg_bf.astype(np.float32) * one_minus_beta[:, None]).astype(np.float32)

    key = tuple(counts)
    if key not in _CACHE:
        _CACHE[key] = _build_nc(blk_rels, counts)
    nc = _CACHE[key]

    wk = {
        'wkv1': _bf(W['kv1'].reshape(2 * 128, 256)),
        'wqp1': _bf(W['qp1'].reshape(3 * 128, 128)),
        'wmb1': _bf(W['mb1'].reshape(3 * 128, 128)),
        'wout1': _bf(W['out1'].reshape(2 * 128, 128)),
        'wkv2': _bf(W['kv2'].reshape(2 * 128, 128)),
        'wqp2': _bf(W['qp2'].reshape(3 * 128, 64)),
        'wmb2': _bf(W['mb2'].reshape(3 * 64, 64)),
        'wout2': _bf(W['out2'].reshape(2 * 64, 64)),
    }
    in_maps = []
    for c in range(NCORES):
        m = dict(wk)
        m['xT'] = xT
        m['xTown'] = np.ascontiguousarray(xT[c * OWN:(c + 1) * OWN])
        m['xrows'] = np.ascontiguousarray(xrows_folded[c * OWN:(c + 1) * OWN])
        m['gsrc'] = gsrc[c]
        m['dloc'] = dloc[c]
        in_maps.append(m)

    import os
    trace = bool(os.environ.get("HGT_TRACE"))
    res = run_bass_kernel_spmd(nc, in_maps, core_ids=list(range(NCORES)), trace=trace)
    global LAST_RESULT
    LAST_RESULT = res

    out_full = np.concatenate([res.results[c]['out'] for c in range(NCORES)], axis=0)
    o_b = out_full[_g_b(np.arange(NB))].astype(np.float32)
    o_c = out_full[_g_c(np.arange(NC))].astype(np.float32)
    return o_b, o_c


# revision 14
# speedup vs baseline: 1.1083x; 1.0708x over previous
"""Trainium2 Bass kernel for nn_CompetitionHGT (2-layer HGT, 60k nodes, 800k edges).

Strategy: destination-sharded edge parallelism across 8 NeuronCores.
 - Nodes padded to 128-blocks, reordered core-major; core k owns 49 b-blocks + 10 c-blocks.
 - Edges sorted by destination, grouped per (core, dst-block, relation), padded to
   128-edge chunks with uniform chunk counts across cores (single SPMD program).
 - Per layer: replicated k|v projection into a bf16 table (per-relation transform
   folded into q-side weights: q.(k@A) = (q@A^T).k, with p_rel*scale folded in);
   edge phase gathers kv rows by src via indirect DMA, builds per-chunk one-hot
   (dst-within-block) matrices, expands q' per edge via PE matmul, computes
   exp(logits), payload v*exp, and scatter-adds via one-hot matmuls into PSUM per
   (block, relation); epilogue normalizes by the joint softmax sum, applies m_rel,
   gelu, typed output linear and gated skip. Chunks are processed in groups of 4
   to amortize vector-engine instruction overhead.
 - Between layers: AllGather of transposed h blocks (bf16).
"""
import sys
if '/opt/trn_rl_repo' not in sys.path:
    sys.path.insert(0, '/opt/trn_rl_repo')

import numpy as np
import ml_dtypes
from contextlib import ExitStack

import concourse.bass as bass
import concourse.bacc as bacc
import concourse.tile as tile
from concourse import mybir
from concourse.bass_utils import run_bass_kernel_spmd
from concourse.masks import make_identity

BF = ml_dtypes.bfloat16
F32, BF16, I32 = mybir.dt.float32, mybir.dt.bfloat16, mybir.dt.int32
AOT = mybir.AluOpType
ACTF = mybir.ActivationFunctionType

NB, NC = 50000, 10000
NBP, NCP = 50176, 10240
BB, CB = NBP // 128, NCP // 128          # 392, 80
NCORES = 8
BBC, CBC = BB // NCORES, CB // NCORES    # 49, 10
NBLK = BBC + CBC                         # 59 blocks per core
OWN = NBLK * 128                         # 7552 rows per core
NTOT = NCORES * OWN                      # 60416
P = 128
G = 4                                    # chunks per group

LAST_RESULT = None


def _g_b(n):
    n = np.asarray(n)
    return (n // (BBC * 128)) * OWN + (n % (BBC * 128))


def _g_c(m):
    m = np.asarray(m)
    return (m // (CBC * 128)) * OWN + BBC * 128 + (m % (CBC * 128))


def _bf(x):
    return np.ascontiguousarray(np.asarray(x, np.float32).astype(BF))


def _prep_edges(e_bb_src, e_bb_dst, e_bc_src, e_bc_dst, e_cb_src, e_cb_dst):
    rels = [(_g_b, 'b', e_bb_src, e_bb_dst),
            (_g_b, 'c', e_bc_src, e_bc_dst),
            (_g_c, 'b', e_cb_src, e_cb_dst)]
    buckets = [[[None] * 3 for _ in range(NBLK)] for _ in range(NCORES)]
    for r, (gsrc_fn, dspace, src, dst) in enumerate(rels):
        order = np.argsort(dst, kind='stable')
        src, dst = np.asarray(src)[order], np.asarray(dst)[order]
        gsrc = gsrc_fn(src)
        if dspace == 'b':
            core = dst // (BBC * 128)
            lblk = (dst % (BBC * 128)) // 128
        else:
            core = dst // (CBC * 128)
            lblk = BBC + (dst % (CBC * 128)) // 128
        dloc = (dst % 128).astype(np.float32)
        key = core.astype(np.int64) * 64 + lblk
        bounds = np.searchsorted(key, np.arange(NCORES * 64 + 64))
        for c in range(NCORES):
            for lb in range(NBLK):
                if (lb < BBC) != (dspace == 'b'):
                    continue
                k = c * 64 + lb
                buckets[c][lb][r] = (gsrc[bounds[k]:bounds[k + 1]],
                                     dloc[bounds[k]:bounds[k + 1]])

    blk_rels = []
    for lb in range(BBC):
        blk_rels += [(lb, 0), (lb, 2)]
    for lb in range(BBC, NBLK):
        blk_rels.append((lb, 1))
    counts = []
    for (lb, r) in blk_rels:
        mx = max(len(buckets[c][lb][r][0]) for c in range(NCORES))
        counts.append(max(1, (mx + P - 1) // P))
    nch = sum(counts)

    gsrc_a = np.zeros((NCORES, nch, P), np.int32)
    dloc_a = np.full((NCORES, nch, P), 255.0, np.float32)
    for c in range(NCORES):
        col = 0
        for (lb, r), cc in zip(blk_rels, counts):
            gs, dl = buckets[c][lb][r]
            n = len(gs)
            fg = np.zeros(cc * P, np.int32); fg[:n] = gs
            fd = np.full(cc * P, 255.0, np.float32); fd[:n] = dl
            gsrc_a[c, col:col + cc] = fg.reshape(cc, P)
            dloc_a[c, col:col + cc] = fd.reshape(cc, P)
            col += cc
    return (blk_rels, counts,
            np.ascontiguousarray(gsrc_a.transpose(0, 2, 1)),
            np.ascontiguousarray(dloc_a.transpose(0, 2, 1).astype(BF)))


def _fold_weights(inp):
    W = {}
    for L, (H, ch) in (('1', (4, 128)), ('2', (1, 64))):
        D = ch // H
        Wkqv = np.asarray(inp[f'l{L}_Wkqv'], np.float32)
        arel = np.asarray(inp[f'l{L}_arel'], np.float32)
        mrel = np.asarray(inp[f'l{L}_mrel'], np.float32)
        prel = np.asarray(inp[f'l{L}_prel'], np.float32)
        scale = np.float32(1.0 / np.sqrt(np.float32(D)))
        Wk, Wq, Wv = Wkqv[:, :, :ch], Wkqv[:, :, ch:2 * ch], Wkqv[:, :, 2 * ch:]
        W[f'kv{L}'] = np.concatenate([Wk, Wv], axis=2)
        qp, mb = [], []
        for r, dt in ((0, 0), (1, 1), (2, 0)):
            blk = np.zeros((ch, ch), np.float32)
            mblk = np.zeros((ch, ch), np.float32)
            for h in range(H):
                blk[h * D:(h + 1) * D, h * D:(h + 1) * D] = arel[r, h].T * (prel[r, h] * scale)
                mblk[h * D:(h + 1) * D, h * D:(h + 1) * D] = mrel[r, h]
            qp.append(Wq[dt] @ blk)
            mb.append(mblk)
        W[f'qp{L}'] = np.stack(qp)
        W[f'mb{L}'] = np.stack(mb)
        beta = 1.0 / (1.0 + np.exp(-np.asarray(inp[f'l{L}_skip'], np.float64)))
        W[f'beta{L}'] = beta.astype(np.float32)
        Wout = np.asarray(inp[f'l{L}_Wout'], np.float32)
        W[f'out{L}'] = Wout * beta[:, None, None].astype(np.float32) if L == '1' else Wout
    return W


def _build_nc(blk_rels, counts):
    nch = sum(counts)
    per_block = {}
    idx = 0
    for (lb, r), cc in zip(blk_rels, counts):
        per_block.setdefault(lb, []).append((r, idx, cc))
        idx += cc

    nc = bacc.Bacc("TRN2", target_bir_lowering=False, debug=False, num_devices=NCORES)

    xTown = nc.dram_tensor("xTown", [OWN, 128], BF16, kind="ExternalInput")
    xrows = nc.dram_tensor("xrows", [OWN, 128], F32, kind="ExternalInput")
    gsrc_d = nc.dram_tensor("gsrc", [P, nch], I32, kind="ExternalInput")
    dloc_d = nc.dram_tensor("dloc", [P, nch], BF16, kind="ExternalInput")
    wkv1 = nc.dram_tensor("wkv1", [2 * 128, 256], BF16, kind="ExternalInput")
    wqp1 = nc.dram_tensor("wqp1", [3 * 128, 128], BF16, kind="ExternalInput")
    wmb1 = nc.dram_tensor("wmb1", [3 * 128, 128], BF16, kind="ExternalInput")
    wout1 = nc.dram_tensor("wout1", [2 * 128, 128], BF16, kind="ExternalInput")
    wkv2 = nc.dram_tensor("wkv2", [2 * 128, 128], BF16, kind="ExternalInput")
    wqp2 = nc.dram_tensor("wqp2", [3 * 128, 64], BF16, kind="ExternalInput")
    wmb2 = nc.dram_tensor("wmb2", [3 * 64, 64], BF16, kind="ExternalInput")
    wout2 = nc.dram_tensor("wout2", [2 * 64, 64], BF16, kind="ExternalInput")
    out_d = nc.dram_tensor("out", [OWN, 64], F32, kind="ExternalOutput")

    with tile.TileContext(nc) as tc:
        with ExitStack() as ctx:
            const = ctx.enter_context(tc.tile_pool(name="const", bufs=1))
            sb = ctx.enter_context(tc.tile_pool(name="sb", bufs=6))
            sbk = ctx.enter_context(tc.tile_pool(name="sbk", bufs=6))
            sbg = ctx.enter_context(tc.tile_pool(name="sbg", bufs=18))
            pp_nd = ctx.enter_context(tc.tile_pool(name="ppnd", bufs=3, space="PSUM"))
            pp_acc = ctx.enter_context(tc.tile_pool(name="ppacc", bufs=3, space="PSUM"))
            pp_epi = ctx.enter_context(tc.tile_pool(name="ppepi", bufs=2, space="PSUM"))
            dram = ctx.enter_context(tc.tile_pool(name="dram", bufs=1, space="DRAM"))

            kv1_own = dram.tile([OWN, 256], BF16, tag="kv1o")
            kv2_own = dram.tile([OWN, 128], BF16, tag="kv2o")
            kv1 = dram.tile([NTOT, 256], BF16, tag="kv1", addr_space="Shared")
            kv2 = dram.tile([NTOT, 128], BF16, tag="kv2", addr_space="Shared")
            hT_own = dram.tile([OWN, 128], BF16, tag="hTown")

            iota_i = const.tile([P, G * P], I32, tag="iotai")
            nc.gpsimd.iota(iota_i[:], pattern=[[0, G], [1, P]], base=0,
                           channel_multiplier=0)
            iota4 = const.tile([P, G, P], BF16, tag="iota4")
            nc.vector.tensor_copy(out=iota4[:].rearrange("p k d -> p (k d)"),
                                  in_=iota_i[:])
            ident = const.tile([P, P], BF16, tag="ident")
            make_identity(nc, ident[:])

            def wtile(dram_t, rows, cols, tag, row0=0):
                t = const.tile([rows, cols], BF16, tag=tag)
                nc.sync.dma_start(out=t[:], in_=dram_t[row0:row0 + rows, :])
                return t

            wkv1_t = [wtile(wkv1, 128, 256, f"wkv1{t}", t * 128) for t in range(2)]
            wqp1_t = [wtile(wqp1, 128, 128, f"wqp1{r}", r * 128) for r in range(3)]
            wmb1_t = [wtile(wmb1, 128, 128, f"wmb1{r}", r * 128) for r in range(3)]
            wout1_t = [wtile(wout1, 128, 128, f"wout1{t}", t * 128) for t in range(2)]
            wkv2_t = [wtile(wkv2, 128, 128, f"wkv2{t}", t * 128) for t in range(2)]
            wqp2_t = [wtile(wqp2, 128, 64, f"wqp2{r}", r * 128) for r in range(3)]
            wmb2_t = [wtile(wmb2, 64, 64, f"wmb2{r}", r * 64) for r in range(3)]
            wout2_t = [wtile(wout2, 64, 64, f"wout2{t}", t * 64) for t in range(2)]

            gsrc_sb = const.tile([P, nch], I32, tag="gsrcsb")
            nc.sync.dma_start(out=gsrc_sb[:], in_=gsrc_d[:])
            dloc_sb = const.tile([P, nch], BF16, tag="dlocsb")
            nc.sync.dma_start(out=dloc_sb[:], in_=dloc_d[:])

            def blk_type(lb):
                return 0 if lb < BBC else 1

            def node_phase(src_own, kv_own, wkv_t, kvcols):
                NG = 512 // kvcols   # blocks per psum bank (2 for L1, 4 for L2)
                for g0 in range(0, NBLK, NG):
                    ng = min(NG, NBLK - g0)
                    xt = sb.tile([P, NG, 128], BF16, tag="nd_x")
                    nc.sync.dma_start(
                        out=xt[:, :ng, :],
                        in_=src_own[g0 * 128:(g0 + ng) * 128, :].rearrange(
                            "(k p) d -> p k d", p=P))
                    ps = pp_nd.tile([P, NG, kvcols], F32, tag="nd_ps", space="PSUM")
                    for j in range(ng):
                        nc.tensor.matmul(out=ps[:, j, :], lhsT=xt[:, j, :],
                                         rhs=wkv_t[blk_type(g0 + j)][:],
                                         start=True, stop=True)
                    kvb = sb.tile([P, NG, kvcols], BF16, tag="nd_kv")
                    nc.vector.tensor_copy(out=kvb[:, :ng, :], in_=ps[:, :ng, :])
                    nc.scalar.dma_start(
                        out=kv_own[g0 * 128:(g0 + ng) * 128, :].rearrange(
                            "(k p) d -> p k d", p=P),
                        in_=kvb[:, :ng, :])

            def edge_phase(layer, kv_tab, src_own, wqp_t, wmb_t, wout_t, H, ch):
                D = ch // H
                F = ch + H
                for lb in range(NBLK):
                    typ = blk_type(lb)
                    xo = sb.tile([P, 128], BF16, tag="e_xo")
                    nc.sync.dma_start(out=xo[:], in_=src_own[lb * 128:(lb + 1) * 128, :])
                    accs = []
                    for (r, cstart, cc) in per_block[lb]:
                        qp_ps = pp_nd.tile([P, 512], F32, tag="nd_ps", space="PSUM")
                        nc.tensor.matmul(out=qp_ps[:, :ch], lhsT=xo[:], rhs=wqp_t[r][:],
                                         start=True, stop=True)
                        qp_sb = sb.tile([P, ch], BF16, tag="e_qp")
                        nc.vector.tensor_copy(out=qp_sb[:], in_=qp_ps[:, :ch])
                        acc = pp_acc.tile([P, F], F32, tag="acc", space="PSUM")
                        gsizes = [G] * (cc // G) + ([cc % G] if cc % G else [])
                        ngrp = len(gsizes)
                        gcol = 0
                        for gi, gs in enumerate(gsizes):
                            col = cstart + gcol
                            gcol += gs
                            kv4 = sbg.tile([P, G, 2 * ch], BF16, tag="e_kv")
                            for j in range(gs):
                                nc.gpsimd.indirect_dma_start(
                                    out=kv4[:, j, :], out_offset=None, in_=kv_tab[:],
                                    in_offset=bass.IndirectOffsetOnAxis(
                                        ap=gsrc_sb[:, col + j:col + j + 1], axis=0))
                            ohT = sbk.tile([P, G, P], BF16, tag="e_oh")
                            nc.vector.tensor_tensor(
                                out=ohT[:, :gs, :], in0=iota4[:, :gs, :],
                                in1=dloc_sb[:, col:col + gs].to_broadcast([P, gs, P]),
                                op=AOT.is_equal)
                            oh_ps = pp_nd.tile([P, G, P], BF16, tag="nd_ps", space="PSUM")
                            for j in range(gs):
                                nc.tensor.transpose(out=oh_ps[:, j, :], in_=ohT[:, j, :],
                                                    identity=ident[:])
                            oh_sb = sbk.tile([P, G, P], BF16, tag="e_ohs")
                            nc.vector.tensor_copy(
                                out=oh_sb[:, :gs, :], in_=oh_ps[:, :gs, :])
                            qe_ps = pp_nd.tile([P, G, ch], F32, tag="nd_ps", space="PSUM")
                            for j in range(gs):
                                nc.tensor.matmul(out=qe_ps[:, j, :], lhsT=oh_sb[:, j, :],
                                                 rhs=qp_sb[:], start=True, stop=True)
                            prod = sbk.tile([P, G, ch], BF16, tag="e_pr")
                            nc.vector.tensor_tensor(
                                out=prod[:, :gs, :], in0=qe_ps[:, :gs, :],
                                in1=kv4[:, :gs, :ch], op=AOT.mult)
                            lgt = sbk.tile([P, G, H], F32, tag="e_lg")
                            nc.vector.reduce_sum(
                                out=lgt[:, :gs, :],
                                in_=prod[:, :gs, :].rearrange("p k (h d) -> p k h d", h=H),
                                axis=mybir.AxisListType.X)
                            pay = sbk.tile([P, G, F], BF16, tag="e_pay")
                            nc.scalar.activation(out=pay[:, :gs, ch:F], in_=lgt[:, :gs, :],
                                                 func=ACTF.Exp)
                            nc.vector.tensor_tensor(
                                out=pay[:, :gs, :ch].rearrange("p k (h d) -> p k h d", h=H),
                                in0=kv4[:, :gs, ch:2 * ch].rearrange("p k (h d) -> p k h d", h=H),
                                in1=pay[:, :gs, ch:F].to_broadcast([P, gs, H, D]),
                                op=AOT.mult)
                            for j in range(gs):
                                nc.tensor.matmul(
                                    out=acc[:], lhsT=ohT[:, j, :], rhs=pay[:, j, :],
                                    start=(gi == 0 and j == 0),
                                    stop=(gi == ngrp - 1 and j == gs - 1))
                        accs.append((r, acc))
                    # epilogue
                    s = sb.tile([P, H], F32, tag="ep_s")
                    nc.vector.tensor_copy(out=s[:], in_=accs[0][1][:, ch:F])
                    if len(accs) > 1:
                        nc.vector.tensor_tensor(out=s[:], in0=accs[1][1][:, ch:F],
                                                in1=s[:], op=AOT.add)
                    nc.vector.tensor_scalar(out=s[:], in0=s[:], scalar1=1e-16,
                                            scalar2=None, op0=AOT.add)
                    recip = sb.tile([P, H], F32, tag="ep_r")
                    nc.vector.reciprocal(out=recip[:], in_=s[:])
                    aggT = pp_epi.tile([ch, P], F32, tag="ep", space="PSUM")
                    for j, (r, acc) in enumerate(accs):
                        vn = sb.tile([P, ch], BF16, tag="ep_vn")
                        nc.vector.tensor_tensor(
                            out=vn[:].rearrange("p (h d) -> p h d", h=H),
                            in0=acc[:, :ch].rearrange("p (h d) -> p h d", h=H),
                            in1=recip[:].to_broadcast([P, H, D]), op=AOT.mult)
                        vT_ps = pp_epi.tile([ch, P], BF16, tag="ep", space="PSUM")
                        nc.tensor.transpose(out=vT_ps[:], in_=vn[:], identity=ident[:])
                        vT = sb.tile([ch, P], BF16, tag="ep_vts")
                        nc.vector.tensor_copy(out=vT[:], in_=vT_ps[:])
                        nc.tensor.matmul(out=aggT[:], lhsT=wmb_t[r][:], rhs=vT[:],
                                         start=(j == 0), stop=(j == len(accs) - 1))
                    gl = sb.tile([ch, P], BF16, tag="ep_gl")
                    nc.scalar.activation(out=gl[:], in_=aggT[:], func=ACTF.Gelu)
                    ops = pp_epi.tile([P, ch if layer == 1 else 64], F32, tag="ep",
                                      space="PSUM")
                    nc.tensor.matmul(out=ops[:], lhsT=gl[:], rhs=wout_t[typ][:],
                                     start=True, stop=True)
                    if layer == 1:
                        xr = sb.tile([P, 128], F32, tag="ep_xr")
                        nc.sync.dma_start(out=xr[:], in_=xrows[lb * 128:(lb + 1) * 128, :])
                        hrow = sb.tile([P, 128], BF16, tag="ep_hr")
                        nc.vector.tensor_tensor(out=hrow[:], in0=ops[:], in1=xr[:],
                                                op=AOT.add)
                        hT_ps = pp_epi.tile([P, P], BF16, tag="ep", space="PSUM")
                        nc.tensor.transpose(out=hT_ps[:], in_=hrow[:], identity=ident[:])
                        hT = sb.tile([P, P], BF16, tag="ep_hts")
                        nc.vector.tensor_scalar(out=hT[:], in0=hT_ps[:], scalar1=0.0,
                                                scalar2=None, op0=AOT.max)
                        nc.scalar.dma_start(out=hT_own[lb * 128:(lb + 1) * 128, :], in_=hT[:])
                    else:
                        fin = sb.tile([P, 64], F32, tag="ep_fin")
                        nc.vector.tensor_copy(out=fin[:], in_=ops[:])
                        nc.scalar.dma_start(out=out_d[lb * 128:(lb + 1) * 128, :], in_=fin[:])

            # ---- layer 1 ----
            node_phase(xTown, kv1_own, wkv1_t, 256)
            nc.gpsimd.collective_compute(
                "AllGather", AOT.bypass, replica_groups=[list(range(NCORES))],
                ins=[kv1_own.opt()], outs=[kv1.opt()])
            edge_phase(1, kv1, xTown, wqp1_t, wmb1_t, wout1_t, 4, 128)
            # ---- layer 2 ----
            node_phase(hT_own, kv2_own, wkv2_t, 128)
            nc.gpsimd.collective_compute(
                "AllGather", AOT.bypass, replica_groups=[list(range(NCORES))],
                ins=[kv2_own.opt()], outs=[kv2.opt()])
            edge_phase(2, kv2, hT_own, wqp2_t, wmb2_t, wout2_t, 1, 64)

    nc.compile()
    return nc


_CACHE = {}


def kernel(**inputs):
    inp = {k: np.asarray(v) for k, v in inputs.items()}

    blk_rels, counts, gsrc, dloc = _prep_edges(
        inp['e_bb_src'], inp['e_bb_dst'], inp['e_bc_src'], inp['e_bc_dst'],
        inp['e_cb_src'], inp['e_cb_dst'])
    W = _fold_weights(inp)

    xg = np.zeros((NTOT, 128), np.float32)
    xb_pad = np.zeros((NBP, 128), np.float32); xb_pad[:NB] = inp['x_b']
    xc_pad = np.zeros((NCP, 128), np.float32); xc_pad[:NC] = inp['x_c']
    xg[_g_b(np.arange(NBP))] = xb_pad
    xg[_g_c(np.arange(NCP))] = xc_pad
    xg_bf = _bf(xg)
    xT = np.ascontiguousarray(
        xg_bf.reshape(NCORES * NBLK, 128, 128).transpose(0, 2, 1)).reshape(NTOT, 128)
    one_minus_beta = np.empty(NTOT, np.float32)
    for c in range(NCORES):
        one_minus_beta[c * OWN:c * OWN + BBC * 128] = 1.0 - W['beta1'][0]
        one_minus_beta[c * OWN + BBC * 128:(c + 1) * OWN] = 1.0 - W['beta1'][1]
    xrows_folded = (xg_bf.astype(np.float32) * one_minus_beta[:, None]).astype(np.float32)

    key = tuple(counts)
    if key not in _CACHE:
        _CACHE[key] = _build_nc(blk_rels, counts)
    nc = _CACHE[key]

    wk = {
        'wkv1': _bf(W['kv1'].reshape(2 * 128, 256)),
        'wqp1': _bf(W['qp1'].reshape(3 * 128, 128)),
        'wmb1': _bf(W['mb1'].reshape(3 * 128, 128)),
        'wout1': _bf(W['out1'].reshape(2 * 128, 128)),
        'wkv2': _bf(W['kv2'].reshape(2 * 128, 128)),
        'wqp2': _bf(W['qp2'].reshape(3 * 128, 64)),
        'wmb2': _bf(W['mb2'].reshape(3 * 64, 64)),
        'wout2': _bf(W['out2'].reshape(2 * 64, 64)),
    }
    in_maps = []
    for c in range(NCORES):
        m = dict(wk)
        m['xTown'] = np.ascontiguousarray(xT[c * OWN:(c + 1) * OWN])
        m['xrows'] = np.ascontiguousarray(xrows_folded[c * OWN:(c + 1) * OWN])
        m['gsrc'] = gsrc[c]
        m['dloc'] = dloc[c]
        in_maps.append(m)

    import os
    trace = bool(os.environ.get("HGT_TRACE"))
    res = run_bass_kernel_spmd(nc, in_maps, core_ids=list(range(NCORES)), trace=trace)
    global LAST_RESULT
    LAST_RESULT = res

    out_full = np.concatenate([res.results[c]['out'] for c in range(NCORES)], axis=0)
    o_b = out_full[_g_b(np.arange(NB))].astype(np.float32)
    o_c = out_full[_g_c(np.arange(NC))].astype(np.float32)
    return o_b, o_c
